# revision 20
# baseline (speedup 1.0000x reference)
"""Chamfer distance kernel for Trainium2 (8 NeuronCores, SPMD).

Problem: xyz1 [4, 8192, 3], xyz2 [4, 8192, 3] (fp32) ->
    scalar = mean_i min_j |x_i - y_j|^2  +  mean_j min_i |x_i - y_j|^2
(means taken over all batches).

Sharding: 8 cores = 4 batches x 2 halves of the N (xyz1-row) dimension.
Core c handles batch c//2, rows [(c%2)*4096, (c%2+1)*4096) of xyz1 and all
8192 rows of xyz2 for that batch.

Per core, the [4096, 8192] squared-distance matrix is produced by the
TensorEngine as a single K=13 fp16 matmul per tile:
    d_ij = x_i . (-2 y_j) + |x_i|^2 * 1 + 1 * |y_j|^2
Every fp32 operand is split into fp16 hi+lo halves (a = ah + al with
ah = fp16(a)); each x.t coordinate product uses the three dominant terms
xh*th + xh*tl + xl*th (the dropped xl*tl is ~2^-22 relative), and the
norm rows are carried as hi+lo against a row of ones.  This runs the PE
at full 16-bit stream rate — fp32 matmuls on TRN2 are split by the
compiler into two half-rate passes (measured 4x slower end to end).

Blocks are processed in PAIRS (even j=2k, odd j=2k+1).  The two blocks of
a pair run in DIFFERENT PE row groups (lhsT/rhs replicated at partition
base 0 and 32) so their matmuls execute concurrently in the systolic
array — K=13 only occupies a quarter of the 128 contraction rows.

  PE:   per pair, 16 even + 16 odd matmuls [128,512], interleaved, into
        four [128,1024] PSUM tensors (per-stream ping-pong).
  ACT:  copy PSUM -> SBUF fp16 (all 8 even groups first, then odd): even
        block straight into ship buffer U_k, odd block into S_tmp.
  DVE:  row-min of the even block: fold tree U_k -> W scratch -> rowmins;
        col-min pair fold:  U_k = min(U_k, S_tmp)   (fp16 2x, one op/pair);
        row-min of the odd block: fold tree in-place in S_tmp -> rowmins.
  DMA:  ship U_k [128, 8192] fp16 to DRAM (overlapped; DMA is idle anyway).

Host combines: per-core col-min partial = min over the 16 shipped pair-mins
and their 128 partitions; dist2 = min of the two cores per batch; means in
fp64.  fp16 for the min stages keeps each d to ~5e-4 relative error; the
final means average the (symmetric) rounding noise down to ~1e-5.

Raw Bass with one explicit semaphore wait per instruction — this
toolchain rejects instructions carrying more than one sync wait.
"""

import numpy as np

import concourse.bass as bass
from concourse import mybir
from concourse.bass_utils import run_bass_kernel_spmd

# Problem geometry (hardcoded per contest rules).
B = 4
N = 8192
M = 8192
NCORES = 8
HALF = N // 2            # xyz1 rows per core
P = 128                  # partitions
NBLK = HALF // P         # 32 row blocks per core
NPAIR = NBLK // 2        # 16 block pairs -> 16 shipped col-min buffers
MM_FREE = 512            # matmul free dim (one PSUM bank of fp32)
GRP = 1024               # psum tensor free dim (2 banks, 2 matmuls)
NGRP = M // GRP          # 8 psum groups per block row
KDIM = 13                # 3 coords x 3 split-product terms + 2x2 norm rows
KPAD = 45                # lhsT/rhs partition span: rows 0-12 and 32-44

F32 = mybir.dt.float32
F16 = mybir.dt.float16
MIN = mybir.AluOpType.min

NUBUF = 3                # ship-buffer ring (ACT write / DVE min / DMA out)
NSBUF = 3                # S_tmp ring

_CACHED_NC = None


def _build_nc():
    from contextlib import ExitStack

    nc = bass.Bass("TRN2", target_bir_lowering=False, debug=False)

    lhsT_d = nc.dram_tensor("lhsT5", [KPAD, HALF], F16, kind="ExternalInput")
    rhs_d = nc.dram_tensor("rhs5", [KPAD, M], F16, kind="ExternalInput")
    rowmins_d = nc.dram_tensor("rowmins", [P, NBLK], F32, kind="ExternalOutput")
    colmin_d = nc.dram_tensor("colmin", [NPAIR, P, M], F16, kind="ExternalOutput")

    with ExitStack() as ctx:
        ec = ctx.enter_context
        lhsT = ec(nc.sbuf_tensor([KPAD, HALF], F16))
        rhs = ec(nc.sbuf_tensor([KPAD, M], F16))
        u_bufs = [
            ec(nc.sbuf_tensor(f"u{i}", [P, M], F16)) for i in range(NUBUF)
        ]
        s_tmp = [
            ec(nc.sbuf_tensor(f"s{i}", [P, M], F16)) for i in range(NSBUF)
        ]
        w = ec(nc.sbuf_tensor([P, M // 2], F16))
        rowmins = ec(nc.sbuf_tensor([P, NBLK], F32))
        # four 2-bank PSUM tensors: per-stream ping-pong (even/odd block)
        pe_ps = [ec(nc.psum_tensor(f"pe{i}", [P, GRP], F32)) for i in range(2)]
        po_ps = [ec(nc.psum_tensor(f"po{i}", [P, GRP], F32)) for i in range(2)]
        dma_sem = ec(nc.semaphore())
        pe_sem = ec(nc.semaphore())
        act_sem = ec(nc.semaphore())
        dve_sem = ec(nc.semaphore())
        out_sem = ec(nc.semaphore())
        block = ec(nc.Block())

        # Orders and counters
        # PE issue order (per pair):   E0 O0 E1 O1 ... E7 O7   (glin = 16k+2c+odd)
        #   pe_sem += 1 after each group's 2nd matmul -> pe_sem = glin+1
        # ACT copy order matches PE production order (same glin indexing) so
        # the in-order PE queue never head-of-line blocks: act_sem += 1/copy.
        # A PSUM tensor is reused by group glin once copy glin-4 retired.
        # dve_sem: +1 after pair colmin (U_k final), +1 after odd rowmin
        # out_sem: +16 per shipped U_k

        @block.sync
        def _(sync):
            sync.dma_start(out=lhsT[:], in_=lhsT_d.ap()).then_inc(dma_sem, 16)
            sync.dma_start(out=rhs[:], in_=rhs_d.ap()).then_inc(dma_sem, 16)
            for k in range(NPAIR):
                sync.wait_ge(dve_sem, 2 * k + 1)
                sync.dma_start(
                    out=colmin_d.ap()[k], in_=u_bufs[k % NUBUF][:]
                ).then_inc(out_sem, 16)
            sync.wait_ge(dve_sem, 2 * NPAIR)
            sync.dma_start(out=rowmins_d.ap(), in_=rowmins[:]).then_inc(dma_sem, 16)

        @block.tensor
        def _(tensor):
            tensor.wait_ge(dma_sem, 32)
            for k in range(NPAIR):
                for c in range(NGRP):
                    for odd in (0, 1):
                        j = 2 * k + odd
                        glin = 16 * k + 2 * c + odd
                        # buffer reuse: the group 4 slots back used this
                        # psum tensor; its copy is act count glin-3
                        if glin >= 4:
                            tensor.wait_ge(act_sem, glin - 3)
                        pt = (po_ps if odd else pe_ps)[c % 2]
                        kb = 32 * odd  # row-group base partition
                        mm = None
                        for t in range(GRP // MM_FREE):
                            mcol = c * GRP + t * MM_FREE
                            mm = nc.tensor.matmul(
                                pt[:, t * MM_FREE:(t + 1) * MM_FREE],
                                lhsT[kb:kb + KDIM, j * P:(j + 1) * P],
                                rhs[kb:kb + KDIM, mcol:mcol + MM_FREE],
                                start=True,
                                stop=True,
                            )
                        mm.then_inc(pe_sem, 1)

        @block.scalar
        def _(scalar):
            for k in range(NPAIR):
                for c in range(NGRP):
                    for odd in (0, 1):
                        if c == 0:
                            if odd == 0 and k >= NUBUF:
                                # U ring slot free once pair k-NUBUF shipped
                                scalar.wait_ge(out_sem, 16 * (k - NUBUF + 1))
                            if odd == 1 and k >= NSBUF:
                                # S slot free after pair k-NSBUF's odd rowmin
                                scalar.wait_ge(dve_sem, 2 * (k - NSBUF + 1))
                        dst = u_bufs[k % NUBUF] if odd == 0 else s_tmp[k % NSBUF]
                        glin = 16 * k + 2 * c + odd
                        scalar.wait_ge(pe_sem, glin + 1)
                        nc.scalar.copy(
                            out=dst[:, c * GRP:(c + 1) * GRP],
                            in_=((po_ps if odd else pe_ps)[c % 2])[:],
                        ).then_inc(act_sem, 1)

        def rowmin_chain(vector, src, scratch, j):
            """Fold tree: min over the M columns of src -> rowmins[:, j].
            First fold reads src (non-destructively) into scratch; the rest
            fold scratch in place.  fp16 2x mode throughout, final 1x reduce
            at width 256."""
            nc.vector.tensor_tensor(
                out=scratch[:, : M // 2], in0=src[:, : M // 2],
                in1=src[:, M // 2:], op=MIN,
            )
            ww = M // 4
            while ww >= 256:
                nc.vector.tensor_tensor(
                    out=scratch[:, :ww], in0=scratch[:, :ww],
                    in1=scratch[:, ww:2 * ww], op=MIN,
                )
                ww //= 2
            return nc.vector.tensor_reduce(
                out=rowmins[:, j:j + 1], in_=scratch[:, : 2 * ww],
                axis=mybir.AxisListType.X, op=MIN,
            )

        @block.vector
        def _(vector):
            for k in range(NPAIR):
                u = u_bufs[k % NUBUF]
                s = s_tmp[k % NSBUF]
                # even-block row-min, split so the first half-tree starts
                # when only the first 4 even chunks have been copied
                vector.wait_ge(act_sem, 16 * k + 7)
                nc.vector.tensor_tensor(  # min of m-quarters 0,1 -> w[:2048]
                    out=w[:, :2048], in0=u[:, :2048],
                    in1=u[:, 2048:4096], op=MIN,
                )
                vector.wait_ge(act_sem, 16 * k + 15)
                nc.vector.tensor_tensor(  # min of m-quarters 2,3 -> w[2048:]
                    out=w[:, 2048:4096], in0=u[:, 4096:6144],
                    in1=u[:, 6144:8192], op=MIN,
                )
                ww = 2048
                while ww >= 256:
                    nc.vector.tensor_tensor(
                        out=w[:, :ww], in0=w[:, :ww],
                        in1=w[:, ww:2 * ww], op=MIN,
                    )
                    ww //= 2
                nc.vector.tensor_reduce(
                    out=rowmins[:, 2 * k:2 * k + 1], in_=w[:, : 2 * ww],
                    axis=mybir.AxisListType.X, op=MIN,
                )
                # odd block landed in S_tmp
                vector.wait_ge(act_sem, 16 * (k + 1))
                nc.vector.tensor_tensor(
                    out=u[:], in0=u[:], in1=s[:], op=MIN
                ).then_inc(dve_sem, 1)
                rowmin_chain(vector, s, s, 2 * k + 1).then_inc(dve_sem, 1)

    return nc


def _get_nc():
    global _CACHED_NC
    if _CACHED_NC is None:
        _CACHED_NC = _build_nc()
    return _CACHED_NC


def _split16(a):
    """fp32/fp64 -> (hi, lo) fp16 with hi + lo ~= a to ~2^-22."""
    hi = a.astype(np.float16)
    lo = (a - hi.astype(np.float64)).astype(np.float16)
    return hi, lo


def _make_in_maps(xyz1, xyz2):
    xyz1 = np.asarray(xyz1, dtype=np.float32)
    xyz2 = np.asarray(xyz2, dtype=np.float32)
    in_maps = []
    for c in range(NCORES):
        b, h = divmod(c, 2)
        x = xyz1[b, h * HALF:(h + 1) * HALF].astype(np.float64)  # [4096, 3]
        t = -2.0 * xyz2[b].astype(np.float64)                    # [8192, 3]
        xh, xl = _split16(x)
        th, tl = _split16(t)
        nxh, nxl = _split16((x ** 2).sum(1))
        nyh, nyl = _split16(((t / 2.0) ** 2).sum(1))

        lhsT5 = np.zeros((KPAD, HALF), np.float16)
        rhs5 = np.zeros((KPAD, M), np.float16)
        for ci in range(3):
            lhsT5[3 * ci + 0] = xh[:, ci]
            lhsT5[3 * ci + 1] = xh[:, ci]
            lhsT5[3 * ci + 2] = xl[:, ci]
            rhs5[3 * ci + 0] = th[:, ci]
            rhs5[3 * ci + 1] = tl[:, ci]
            rhs5[3 * ci + 2] = th[:, ci]
        lhsT5[9] = nxh
        lhsT5[10] = nxl
        lhsT5[11] = 1.0
        lhsT5[12] = 1.0
        rhs5[9] = 1.0
        rhs5[10] = 1.0
        rhs5[11] = nyh
        rhs5[12] = nyl
        # replicate the K rows at partition base 32 for the odd row group
        lhsT5[32:45] = lhsT5[0:13]
        rhs5[32:45] = rhs5[0:13]
        in_maps.append({"lhsT5": lhsT5, "rhs5": rhs5})
    return in_maps


def _combine(results):
    # mean over all row-min (dist1) values: every core contributes 4096 rows
    d1 = np.stack([np.asarray(r["rowmins"], np.float64) for r in results])
    # per-core col-min partials [16, 128, 8192]: min over pair-bufs and
    # partitions, then min across the two cores sharing each batch
    cm = np.stack(
        [
            np.asarray(r["colmin"]).astype(np.float32).min(axis=(0, 1))
            for r in results
        ]
    )  # [8, 8192]
    dist2 = np.minimum(cm[0::2], cm[1::2]).astype(np.float64)  # [4, 8192]
    return np.float32(d1.mean() + dist2.mean())


def _run(xyz1, xyz2, trace=False):
    nc = _get_nc()
    in_maps = _make_in_maps(xyz1, xyz2)
    res = run_bass_kernel_spmd(nc, in_maps, list(range(NCORES)), trace=trace)
    return _combine(res.results), res


def kernel(xyz1, xyz2):
    out, _ = _run(xyz1, xyz2, trace=False)
    return out
